# revision 4
# baseline (speedup 1.0000x reference)
"""ARAP loss kernel for Trainium2 (8 NeuronCores, SPMD).

Strategy:
  - Per-vertex record table in HBM, bf16, 256B rows: tbl[row, f] with
    f = c*32 + t*16 + b (c: xyz, t: 0=x/1=dx, b: batch), f>=96 zero pad.
    Vertices are split into 4 chunks of 32767 rows (dma_gather indices
    are int16); row 32767 of each chunk is an all-zero record used for
    padding edges (a pad edge gathers zero for both endpoints -> |0|
    contributes nothing).
  - Edges are sharded across the 8 cores. Each core's edges are
    bucket-sorted by (chunk_j, chunk_k) so every 1024-edge instruction
    slot gathers from a single chunk per side (int16 local indices).
    The slot schedule (bucket -> #slots) is shared by all cores.
  - Per slot: two gpsimd.dma_gather (j-records, k-records), 1024 edges
    each. Slots are grouped G at a time into one SBUF tile pair, then
    bf16 DVE compute:
        d = rec_j - rec_k; d2 = d*d
        s(g) = d2[c0] + d2[c1] + d2[c2]        (g = (t,b))
        diff(b) = s(x,b) - s(dx,b)
        acc[p, b] += sum_cols |diff|
  - Host sums [128,16] partials over partitions/cores, divides by E.
"""

import sys

sys.path.insert(0, "/opt/trn_rl_repo")

import numpy as np
import ml_dtypes

import concourse.bass as bass
import concourse.tile as tile
from concourse import bacc, mybir
from concourse.bass_utils import run_bass_kernel_spmd

NV = 100000
B = 16
N_CORES = 8
CHUNK_CAP = 32767          # vertices per chunk (row 32767 = zero record)
N_CHUNKS = 4
SLOT = 1024                # edges per dma_gather (SWDGE ring limit)
G = 8                      # slots per compute group (8192 edges)

_nc_cache = {}


def _build_nc(slot_chunks):
    """slot_chunks: list of (cj, ck) per instruction slot (static schedule)."""
    key = tuple(slot_chunks)
    if key in _nc_cache:
        return _nc_cache[key]

    n_slots = len(slot_chunks)
    assert n_slots % G == 0
    bf16 = mybir.dt.bfloat16
    f32 = mybir.dt.float32

    nc = bacc.Bacc("TRN2", target_bir_lowering=False, debug=False,
                   num_devices=N_CORES)
    tbl_ap = nc.dram_tensor("tbl", [N_CHUNKS * 32768, 128], bf16,
                            kind="ExternalInput").ap()
    idxj_ap = nc.dram_tensor("idxj", [n_slots, 128, SLOT // 16],
                             mybir.dt.int16, kind="ExternalInput").ap()
    idxk_ap = nc.dram_tensor("idxk", [n_slots, 128, SLOT // 16],
                             mybir.dt.int16, kind="ExternalInput").ap()
    out_ap = nc.dram_tensor("out", [128, 16], f32, kind="ExternalOutput").ap()

    W = SLOT // 128  # dst columns per slot (8)

    with tile.TileContext(nc) as tc:
        with tc.tile_pool(name="acc", bufs=1) as acc_pool, \
             tc.tile_pool(name="idx", bufs=2) as idx_pool, \
             tc.tile_pool(name="gat", bufs=2) as gat_pool, \
             tc.tile_pool(name="cmp", bufs=2) as cmp_pool:

            acc = acc_pool.tile([128, 16], f32)
            nc.vector.memset(acc[:], 0.0)

            for grp in range(n_slots // G):
                ij = idx_pool.tile([128, G, SLOT // 16], mybir.dt.int16,
                                   tag="ij")
                nc.scalar.dma_start(
                    ij[:], idxj_ap[grp * G: (grp + 1) * G].rearrange(
                        "g p s -> p g s"))
                ik = idx_pool.tile([128, G, SLOT // 16], mybir.dt.int16,
                                   tag="ik")
                nc.scalar.dma_start(
                    ik[:], idxk_ap[grp * G: (grp + 1) * G].rearrange(
                        "g p s -> p g s"))

                gj = gat_pool.tile([128, G * W, 128], bf16, tag="gj")
                gk = gat_pool.tile([128, G * W, 128], bf16, tag="gk")
                for i in range(G):
                    cj, ck = slot_chunks[grp * G + i]
                    nc.gpsimd.dma_gather(
                        out_ap=gj[:, i * W:(i + 1) * W, :],
                        in_ap=tbl_ap[cj * 32768:(cj + 1) * 32768],
                        idxs_ap=ij[:, i, :],
                        num_idxs=SLOT, num_idxs_reg=SLOT, elem_size=128,
                    )
                    nc.gpsimd.dma_gather(
                        out_ap=gk[:, i * W:(i + 1) * W, :],
                        in_ap=tbl_ap[ck * 32768:(ck + 1) * 32768],
                        idxs_ap=ik[:, i, :],
                        num_idxs=SLOT, num_idxs_reg=SLOT, elem_size=128,
                    )

                M = G * W
                d = cmp_pool.tile([128, M, 96], bf16, tag="d")
                nc.vector.tensor_sub(d[:], gj[:, :, 0:96], gk[:, :, 0:96])
                d2 = cmp_pool.tile([128, M, 96], bf16, tag="d2")
                nc.vector.tensor_mul(d2[:], d[:], d[:])

                s = cmp_pool.tile([128, M, 32], bf16, tag="s")
                nc.vector.tensor_add(s[:], d2[:, :, 0:32], d2[:, :, 32:64])
                nc.vector.tensor_add(s[:], s[:], d2[:, :, 64:96])

                dfx = cmp_pool.tile([128, M, 16], bf16, tag="dfx")
                nc.vector.tensor_sub(dfx[:], s[:, :, 0:16], s[:, :, 16:32])

                red = cmp_pool.tile([128, 16], f32, tag="red")
                nc.vector.tensor_reduce(
                    red[:], dfx[:].rearrange("p e b -> p b e"),
                    axis=mybir.AxisListType.X, op=mybir.AluOpType.add,
                    apply_absolute_value=True,
                )
                nc.vector.tensor_add(acc[:], acc[:], red[:])

            nc.scalar.dma_start(out_ap[:], acc[:])

    nc.finalize()
    _nc_cache[key] = nc
    return nc


def _pack_table(dx, x):
    recs = np.empty((NV, 3, 2, B), dtype=np.float32)
    recs[:, :, 0, :] = x.transpose(1, 2, 0)
    recs[:, :, 1, :] = dx.transpose(1, 2, 0)
    recs = recs.reshape(NV, 96)
    tbl = np.zeros((N_CHUNKS * 32768, 128), dtype=ml_dtypes.bfloat16)
    v = np.arange(NV)
    rows = (v // CHUNK_CAP) * 32768 + v % CHUNK_CAP
    tbl[rows, :96] = recs.astype(ml_dtypes.bfloat16)
    return tbl


def _wrap16(idx_slots):
    """[n_slots, SLOT] -> [n_slots, 128, SLOT//16]: index n at partition
    n%16, column n//16, replicated to all 8 gpsimd cores."""
    n_slots = idx_slots.shape[0]
    w = idx_slots.reshape(n_slots, SLOT // 16, 16).transpose(0, 2, 1)
    return np.tile(w, (1, 8, 1)).astype(np.int16)


def kernel(dx, x, edges):
    E = edges.shape[0]
    tbl = _pack_table(dx, x)

    ej = edges[:, 0].astype(np.int64)
    ek = edges[:, 1].astype(np.int64)
    cj, lj = ej // CHUNK_CAP, ej % CHUNK_CAP
    ck, lk = ek // CHUNK_CAP, ek % CHUNK_CAP
    bucket = cj * N_CHUNKS + ck

    # interleaved sharding keeps bucket distributions similar across cores
    # (edges arrive sorted by j), minimizing shared-schedule padding
    core_j, core_k, core_counts = [], [], []
    for c in range(N_CORES):
        sel = np.arange(c, E, N_CORES)
        order = sel[np.argsort(bucket[sel], kind="stable")]
        bc = np.bincount(bucket[sel], minlength=N_CHUNKS * N_CHUNKS)
        core_j.append(lj[order])
        core_k.append(lk[order])
        core_counts.append(bc)
    core_counts = np.array(core_counts)  # [8, 16]

    # shared slot schedule: per bucket, max over cores of ceil(count/SLOT)
    slots_per_bucket = (-(-core_counts // SLOT)).max(axis=0)  # [16]
    n_slots = int(slots_per_bucket.sum())
    n_slots = -(-n_slots // G) * G  # pad to group multiple
    slot_chunks = []
    for b in range(N_CHUNKS * N_CHUNKS):
        slot_chunks += [(b // N_CHUNKS, b % N_CHUNKS)] * int(slots_per_bucket[b])
    while len(slot_chunks) < n_slots:
        slot_chunks.append((0, 0))

    nc = _build_nc(slot_chunks)

    in_maps = []
    for c in range(N_CORES):
        jj = np.full((n_slots, SLOT), CHUNK_CAP, dtype=np.int64)
        kk = np.full((n_slots, SLOT), CHUNK_CAP, dtype=np.int64)
        pos = 0       # position within this core's sorted stream
        slot0 = 0     # first slot of current bucket
        for b in range(N_CHUNKS * N_CHUNKS):
            cnt = int(core_counts[c, b])
            seg_j = core_j[c][pos:pos + cnt]
            seg_k = core_k[c][pos:pos + cnt]
            flat_j = jj[slot0:slot0 + int(slots_per_bucket[b])].reshape(-1)
            flat_k = kk[slot0:slot0 + int(slots_per_bucket[b])].reshape(-1)
            flat_j[:cnt] = seg_j
            flat_k[:cnt] = seg_k
            pos += cnt
            slot0 += int(slots_per_bucket[b])
        in_maps.append({
            "tbl": tbl,
            "idxj": _wrap16(jj),
            "idxk": _wrap16(kk),
        })

    res = run_bass_kernel_spmd(nc, in_maps, list(range(N_CORES)))

    total = np.zeros(16, dtype=np.float64)
    for c in range(N_CORES):
        total += res.results[c]["out"].astype(np.float64).sum(axis=0)
    return (total / E).astype(np.float32)


# revision 6
# speedup vs baseline: 1.9324x; 1.9324x over previous
"""ARAP loss kernel for Trainium2 (8 NeuronCores, SPMD).

Strategy:
  - Per-vertex record table in HBM, bf16, 256B rows: tbl[row, f] with
    f = c*32 + t*16 + b (c: xyz, t: 0=x/1=dx, b: batch), f>=96 zero pad.
    Vertices are split into 4 chunks of 32767 rows (dma_gather indices
    are int16); row 32767 of each chunk is an all-zero record used for
    padding edges (a pad edge gathers zero for both endpoints -> |0|
    contributes nothing).
  - Edges are sharded across the 8 cores. Each core's edges are
    bucket-sorted by (chunk_j, chunk_k) so every 1024-edge instruction
    slot gathers from a single chunk per side (int16 local indices).
    The slot schedule (bucket -> #slots) is shared by all cores.
  - Per slot: two gpsimd.dma_gather (j-records, k-records), 1024 edges
    each. Slots are grouped G at a time into one SBUF tile pair, then
    bf16 DVE compute:
        d = rec_j - rec_k; d2 = d*d
        s(g) = d2[c0] + d2[c1] + d2[c2]        (g = (t,b))
        diff(b) = s(x,b) - s(dx,b)
        acc[p, b] += sum_cols |diff|
  - Host sums [128,16] partials over partitions/cores, divides by E.
"""

import sys

sys.path.insert(0, "/opt/trn_rl_repo")

import numpy as np
import ml_dtypes

import concourse.bass as bass
import concourse.tile as tile
from concourse import bacc, mybir
from concourse.bass_utils import run_bass_kernel_spmd

NV = 100000
B = 16
N_CORES = 8
CHUNK_CAP = 32767          # vertices per chunk (row 32767 = zero record)
N_CHUNKS = 4
SLOT = 1024                # edges per dma_gather (SWDGE ring limit)
G = 8                      # slots per compute group (8192 edges)

_nc_cache = {}


def _build_nc(slot_chunks):
    """slot_chunks: list of (cj, ck) per instruction slot (static schedule)."""
    key = tuple(slot_chunks)
    if key in _nc_cache:
        return _nc_cache[key]

    n_slots = len(slot_chunks)
    assert n_slots % G == 0
    bf16 = mybir.dt.bfloat16
    f32 = mybir.dt.float32

    nc = bacc.Bacc("TRN2", target_bir_lowering=False, debug=False,
                   num_devices=N_CORES)
    tbl_ap = nc.dram_tensor("tbl", [N_CHUNKS * 32768, 128], bf16,
                            kind="ExternalInput").ap()
    idxj_ap = nc.dram_tensor("idxj", [n_slots, 128, SLOT // 16],
                             mybir.dt.int16, kind="ExternalInput").ap()
    idxk_ap = nc.dram_tensor("idxk", [n_slots, 128, SLOT // 16],
                             mybir.dt.int16, kind="ExternalInput").ap()
    out_ap = nc.dram_tensor("out", [128, 16], f32, kind="ExternalOutput").ap()

    W = SLOT // 128  # dst columns per slot (8)

    with tile.TileContext(nc) as tc:
        with tc.tile_pool(name="acc", bufs=1) as acc_pool, \
             tc.tile_pool(name="idx", bufs=2) as idx_pool, \
             tc.tile_pool(name="gat", bufs=2) as gat_pool, \
             tc.tile_pool(name="cmp", bufs=2) as cmp_pool:

            acc = acc_pool.tile([128, 16], f32)
            nc.vector.memset(acc[:], 0.0)

            for grp in range(n_slots // G):
                ij = idx_pool.tile([128, G, SLOT // 16], mybir.dt.int16,
                                   tag="ij")
                nc.scalar.dma_start(
                    ij[:], idxj_ap[grp * G: (grp + 1) * G].rearrange(
                        "g p s -> p g s"))
                ik = idx_pool.tile([128, G, SLOT // 16], mybir.dt.int16,
                                   tag="ik")
                nc.scalar.dma_start(
                    ik[:], idxk_ap[grp * G: (grp + 1) * G].rearrange(
                        "g p s -> p g s"))

                gj = gat_pool.tile([128, G * W, 128], bf16, tag="gj")
                gk = gat_pool.tile([128, G * W, 128], bf16, tag="gk")
                for i in range(G):
                    cj, ck = slot_chunks[grp * G + i]
                    nc.gpsimd.dma_gather(
                        out_ap=gj[:, i * W:(i + 1) * W, :],
                        in_ap=tbl_ap[cj * 32768:(cj + 1) * 32768],
                        idxs_ap=ij[:, i, :],
                        num_idxs=SLOT, num_idxs_reg=SLOT, elem_size=128,
                    )
                    nc.gpsimd.dma_gather(
                        out_ap=gk[:, i * W:(i + 1) * W, :],
                        in_ap=tbl_ap[ck * 32768:(ck + 1) * 32768],
                        idxs_ap=ik[:, i, :],
                        num_idxs=SLOT, num_idxs_reg=SLOT, elem_size=128,
                    )

                M = G * W
                d = cmp_pool.tile([128, M, 96], bf16, tag="d")
                nc.vector.tensor_sub(d[:], gj[:, :, 0:96], gk[:, :, 0:96])
                d2 = cmp_pool.tile([128, M, 96], bf16, tag="d2")
                nc.vector.tensor_mul(d2[:], d[:], d[:])

                s = cmp_pool.tile([128, M, 32], bf16, tag="s")
                nc.vector.tensor_add(s[:], d2[:, :, 0:32], d2[:, :, 32:64])
                nc.vector.tensor_add(s[:], s[:], d2[:, :, 64:96])

                dfx = cmp_pool.tile([128, M, 16], bf16, tag="dfx")
                nc.vector.tensor_sub(dfx[:], s[:, :, 0:16], s[:, :, 16:32])

                red = cmp_pool.tile([128, 16], f32, tag="red")
                nc.vector.tensor_reduce(
                    red[:], dfx[:].rearrange("p e b -> p b e"),
                    axis=mybir.AxisListType.X, op=mybir.AluOpType.add,
                    apply_absolute_value=True,
                )
                nc.vector.tensor_add(acc[:], acc[:], red[:])

            nc.scalar.dma_start(out_ap[:], acc[:])

    nc.finalize()
    _nc_cache[key] = nc
    return nc


def _pack_table(dx, x):
    recs = np.empty((NV, 3, 2, B), dtype=np.float32)
    recs[:, :, 0, :] = x.transpose(1, 2, 0)
    recs[:, :, 1, :] = dx.transpose(1, 2, 0)
    recs = recs.reshape(NV, 96)
    tbl = np.zeros((N_CHUNKS * 32768, 128), dtype=ml_dtypes.bfloat16)
    v = np.arange(NV)
    rows = (v // CHUNK_CAP) * 32768 + v % CHUNK_CAP
    tbl[rows, :96] = recs.astype(ml_dtypes.bfloat16)
    return tbl


def _wrap16(idx_slots):
    """[n_slots, SLOT] -> [n_slots, 128, SLOT//16]: index n at partition
    n%16, column n//16, replicated to all 8 gpsimd cores."""
    n_slots = idx_slots.shape[0]
    w = idx_slots.reshape(n_slots, SLOT // 16, 16).transpose(0, 2, 1)
    return np.tile(w, (1, 8, 1)).astype(np.int16)


def kernel(dx, x, edges):
    dx = np.asarray(dx, dtype=np.float32)
    x = np.asarray(x, dtype=np.float32)
    edges = np.asarray(edges)
    E = edges.shape[0]
    tbl = _pack_table(dx, x)

    ej = edges[:, 0].astype(np.int64)
    ek = edges[:, 1].astype(np.int64)
    # The reference's unique directed edge list contains (k,j) for every
    # (j,k) (all 6 directed face edges are inserted before dedup) and
    # |diff| is symmetric, so processing j<k once and doubling is exact.
    # Self-loops contribute |0|. Verify symmetry; fall back if violated.
    fwd, bwd = ej < ek, ej > ek
    if np.array_equal(np.sort(ej[fwd] * NV + ek[fwd]),
                      np.sort(ek[bwd] * NV + ej[bwd])):
        ej, ek = ej[fwd], ek[fwd]
        scale = 2.0
    else:
        scale = 1.0
    E_proc = ej.shape[0]
    cj, lj = ej // CHUNK_CAP, ej % CHUNK_CAP
    ck, lk = ek // CHUNK_CAP, ek % CHUNK_CAP
    bucket = cj * N_CHUNKS + ck

    # interleaved sharding keeps bucket distributions similar across cores
    # (edges arrive sorted by j), minimizing shared-schedule padding
    core_j, core_k, core_counts = [], [], []
    for c in range(N_CORES):
        sel = np.arange(c, E_proc, N_CORES)
        order = sel[np.argsort(bucket[sel], kind="stable")]
        bc = np.bincount(bucket[sel], minlength=N_CHUNKS * N_CHUNKS)
        core_j.append(lj[order])
        core_k.append(lk[order])
        core_counts.append(bc)
    core_counts = np.array(core_counts)  # [8, 16]

    # shared slot schedule: per bucket, max over cores of ceil(count/SLOT)
    slots_per_bucket = (-(-core_counts // SLOT)).max(axis=0)  # [16]
    n_slots = int(slots_per_bucket.sum())
    n_slots = -(-n_slots // G) * G  # pad to group multiple
    slot_chunks = []
    for b in range(N_CHUNKS * N_CHUNKS):
        slot_chunks += [(b // N_CHUNKS, b % N_CHUNKS)] * int(slots_per_bucket[b])
    while len(slot_chunks) < n_slots:
        slot_chunks.append((0, 0))

    nc = _build_nc(slot_chunks)

    in_maps = []
    for c in range(N_CORES):
        jj = np.full((n_slots, SLOT), CHUNK_CAP, dtype=np.int64)
        kk = np.full((n_slots, SLOT), CHUNK_CAP, dtype=np.int64)
        pos = 0       # position within this core's sorted stream
        slot0 = 0     # first slot of current bucket
        for b in range(N_CHUNKS * N_CHUNKS):
            cnt = int(core_counts[c, b])
            seg_j = core_j[c][pos:pos + cnt]
            seg_k = core_k[c][pos:pos + cnt]
            flat_j = jj[slot0:slot0 + int(slots_per_bucket[b])].reshape(-1)
            flat_k = kk[slot0:slot0 + int(slots_per_bucket[b])].reshape(-1)
            flat_j[:cnt] = seg_j
            flat_k[:cnt] = seg_k
            pos += cnt
            slot0 += int(slots_per_bucket[b])
        in_maps.append({
            "tbl": tbl,
            "idxj": _wrap16(jj),
            "idxk": _wrap16(kk),
        })

    res = run_bass_kernel_spmd(nc, in_maps, list(range(N_CORES)))

    total = np.zeros(16, dtype=np.float64)
    for c in range(N_CORES):
        total += res.results[c]["out"].astype(np.float64).sum(axis=0)
    return (scale * total / E).astype(np.float32)
